# revision 28
# baseline (speedup 1.0000x reference)
"""CapsuleLayer (dynamic routing, 3 iterations) Trainium2 Bass kernel.

Problem (hardcoded):
    x: [64, 2048, 8] f32, W: [2048, 32, 8, 16] f32
    u_hat[b,o,i,k] = sum_d x[b,i,d] * W[i,o,d,k]
    3 rounds of routing-by-agreement over logits b[B,O,I], softmax over O.
    out v: [64, 32, 16] f32.

Sharding: data-parallel over batch across 8 cores (8 batch rows each), W
replicated. Everything on-chip per core:
  - u_hat computed once on PE via block-diag trick:
      per i-tile of 16: lhsT[(g,d),(g,b)] = x (block-diag), rhs[(g,d),(k,o)] = W
      -> u[(g,b), (k,o)] tiles, stored bf16 in SBUF (16 MiB).
  - round 0: s0 = (1/32) sum_i u_hat via a second accumulating matmul with
      lhsT = x-tile (no block diag) directly from x/W (fp32-exact in PSUM).
  - rounds 1,2: per batch of 8 tiles: vu = u*v (DVE, bf16 2x), agreement =
      k-tree-reduce (DVE), logits update, batched softmax over O (one ACT exp
      per batch + DVE row-sum + recip), cu = u*c (DVE), s += ones-matmul over
      i-partitions (PE).
  - squash + partition broadcast of v via PE ones-matmul.
Free-dim layout is (k, o): column = k*32 + o.
W is DMA'd 4 tiles per transfer (4 KiB/partition lines); PSUM->SBUF copies of
u_hat are done in 2-tile pairs, mostly on the ACT engine (DVE is the round
bottleneck; ACT is otherwise mostly idle).
"""

import numpy as np
import ml_dtypes

BF16 = ml_dtypes.bfloat16

B, I, D, O, K = 64, 2048, 8, 32, 16
NC_N = 8              # cores
BL = B // NC_N        # 8 batch rows per core
G = 16                # i's per tile
T = I // G            # 128 tiles
FREE = O * K          # 512, layout (k,o): col = k*32+o
EPS = 1e-7
BATCH = 16            # tiles per DVE instruction batch in routing rounds
WQ = 4                # W tiles per DMA
ACT_COPY_OF_8 = 5     # of every 8 tile-pair copies, this many go to ACT

_CACHE = {}


def _build_bass():
    import concourse.bass as bass
    import concourse.bacc as bacc
    import concourse.mybir as mybir
    import concourse.tile as tile

    f32 = mybir.dt.float32
    bf16 = mybir.dt.bfloat16
    nc = bacc.Bacc()

    wd = nc.dram_tensor("w", [T // WQ, 128, WQ, FREE], bf16, kind="ExternalInput")
    xtd = nc.dram_tensor("xt", [128, T, BL], bf16, kind="ExternalInput")
    maskd = nc.dram_tensor("mask", [128, 128], bf16, kind="ExternalInput")
    onesd = nc.dram_tensor("ones", [128, BL], bf16, kind="ExternalInput")
    onestd = nc.dram_tensor("onest", [BL, 128], bf16, kind="ExternalInput")
    outd = nc.dram_tensor("out", [BL, FREE], f32, kind="ExternalOutput")

    AX = mybir.AxisListType
    ALU = mybir.AluOpType
    ACTF = mybir.ActivationFunctionType

    with tile.TileContext(nc) as tc:
        with (
            tc.tile_pool(name="const", bufs=1) as constp,
            tc.tile_pool(name="u16", bufs=1) as up,
            tc.tile_pool(name="logits", bufs=1) as blp,
            tc.tile_pool(name="vexp", bufs=1) as vexpp,
            tc.tile_pool(name="psum_s", bufs=1, space="PSUM") as psum_s,
            tc.tile_pool(name="psum_v", bufs=1, space="PSUM") as psum_v,
        ):
            eps_sb = constp.tile([128, 1], f32)
            xt_sb = constp.tile([128, T, BL], bf16)
            ones_sb = constp.tile([128, BL], bf16)
            onest_sb = constp.tile([BL, 128], bf16)

            u16 = up.tile([128, T, FREE], bf16)
            bL = blp.tile([128, T, O], bf16)

            # ---------------- pass 0: u_hat + s0 ----------------
            s0_ps = psum_s.tile([BL, FREE], f32)
            with (
                tc.tile_pool(name="xblk", bufs=1) as xblkp,
                tc.tile_pool(name="wt", bufs=4) as wtp,
                tc.tile_pool(name="psum_u", bufs=2, space="PSUM") as psum_u,
            ):
                # block-diag xblk[g*8+d, t, g*8+b] = x[b, t*16+g, d] built
                # ON-CHIP: broadcast-expand xt over the 16 column groups,
                # then multiply by a [128,128] block-diagonal 0/1 mask
                # (saves 4 MiB of HBM traffic; DVE is idle during pass 0)
                xblk = xblkp.tile([128, T, 128], bf16)
                mask_sb = constp.tile([128, 128], bf16)
                nc.gpsimd.dma_start(xt_sb[:], xtd[:])
                nc.gpsimd.dma_start(mask_sb[:], maskd[:])
                nc.gpsimd.memset(eps_sb[:], EPS)
                nc.gpsimd.dma_start(ones_sb[:], onesd[:])
                nc.gpsimd.dma_start(onest_sb[:], onestd[:])

                def build_xblk(c):
                    sl = slice(16 * c, 16 * (c + 1))
                    nc.vector.tensor_copy(
                        xblk[:, sl, :].rearrange("p t (g b) -> p t g b", g=G),
                        xt_sb[:, sl, :].unsqueeze(2).broadcast_to(
                            [128, 16, G, BL]))
                    nc.vector.tensor_mul(
                        xblk[:, sl, :], xblk[:, sl, :],
                        mask_sb[:].unsqueeze(1).broadcast_to([128, 16, 128]))

                build_xblk(0)
                build_xblk(1)
                npair = 0
                for q in range(T // WQ):
                    c = q // 4 + 2
                    if q % 4 == 0 and c < 8:
                        build_xblk(c)
                    wt = wtp.tile([128, WQ, FREE], bf16)
                    nc.gpsimd.dma_start(wt[:], wd[q])
                    for jj in range(WQ // 2):
                        ut_ps = psum_u.tile([128, 2, FREE], f32)
                        for j2 in range(2):
                            j = 2 * jj + j2
                            t = WQ * q + j
                            nc.tensor.matmul(
                                ut_ps[:, j2, :], xblk[:, t, :], wt[:, j, :])
                            # s0 accumulation straight from x,W
                            nc.tensor.matmul(
                                s0_ps[:], xt_sb[:, t, :], wt[:, j, :],
                                start=(t == 0), stop=(t == T - 1),
                            )
                        # PSUM -> SBUF bf16 cast copy of the pair, ACT-heavy
                        tp = WQ * q + 2 * jj
                        if npair % 8 < ACT_COPY_OF_8:
                            nc.scalar.copy(u16[:, tp:tp + 2, :], ut_ps[:])
                        else:
                            nc.vector.tensor_copy(u16[:, tp:tp + 2, :], ut_ps[:])
                        npair += 1

            # ---------------- squash + broadcast helpers ----------------
            with tc.tile_pool(name="sq", bufs=1) as sqp:

                def squash_and_bcast(s_ps, scale_const, last):
                    """v = squash(s_ps * scale_const); returns vexp1 [128,FREE]
                    (bf16) or DMAs fp32 v to outd if last."""
                    # sq2 = (s_ps*sc)^2 on ACT (Square), straight from PSUM --
                    # keeps the boundary chain on one queue
                    sq2 = sqp.tile([BL, O, K], f32, tag="sq2")
                    nc.scalar.activation(
                        sq2[:],
                        s_ps[:].rearrange("p (k o) -> p o k", o=O),
                        ACTF.Square, scale=float(scale_const))
                    s2 = sqp.tile([BL, O], f32, tag="s2")
                    nc.vector.reduce_sum(s2[:], sq2[:], axis=AX.X)
                    # rt = sqrt(s2+eps) = exp(0.5*ln(s2+eps)): Ln/Exp/Square/
                    # Copy share one ACT function table (Sqrt does not), so no
                    # ACT_TABLE_LOAD lands in the round-boundary chain
                    lns = sqp.tile([BL, O], f32, tag="lns")
                    nc.scalar.activation(lns[:], s2[:], ACTF.Ln, bias=eps_sb[:BL])
                    rt = sqp.tile([BL, O], f32, tag="rt")
                    nc.scalar.activation(rt[:], lns[:], ACTF.Exp, scale=0.5)
                    onep = sqp.tile([BL, O], f32, tag="onep")
                    nc.scalar.add(onep[:], s2[:], 1.0)
                    den = sqp.tile([BL, O], f32, tag="den")
                    nc.vector.tensor_mul(den[:], rt[:], onep[:])
                    rden = sqp.tile([BL, O], f32, tag="rden")
                    nc.vector.reciprocal(rden[:], den[:])
                    # scl = (s2 * sc) * rden -- the s-scale is folded in so v
                    # can read s_ps directly (no staging copy of s)
                    scl = sqp.tile([BL, O], f32, tag="scl")
                    nc.vector.scalar_tensor_tensor(
                        scl[:], s2[:], float(scale_const), rden[:],
                        op0=ALU.mult, op1=ALU.mult)
                    # v = s_ps * scl (broadcast over k)
                    v = sqp.tile([BL, K, O], f32 if last else bf16, tag="v")
                    nc.vector.tensor_mul(
                        v[:], s_ps[:].rearrange("p (k o) -> p k o", o=O),
                        scl[:].unsqueeze(1).broadcast_to([BL, K, O]))
                    if last:
                        nc.gpsimd.dma_start(outd[:], v[:].rearrange("p k o -> p (k o)"))
                        return None
                    # replicate v to all 16 partition groups via PE
                    vrep_ps = psum_v.tile([128, FREE], f32, tag="vrep")
                    nc.tensor.matmul(
                        vrep_ps[:], onest_sb[:],
                        v[:].rearrange("p k o -> p (k o)"))
                    vexp1 = vexpp.tile([128, FREE], bf16, tag="vexp1")
                    nc.scalar.copy(vexp1[:], vrep_ps[:])
                    return vexp1

                vexp1 = squash_and_bcast(s0_ps, 1.0 / O, last=False)

                # ---------------- rounds 1, 2 ----------------
                with (
                    tc.tile_pool(name="rnd", bufs=2) as rp,
                    tc.tile_pool(name="tree", bufs=1) as treep,
                    tc.tile_pool(name="rnd2", bufs=2) as rp2,
                ):
                    for rnd in (1, 2):
                        s_ps = psum_s.tile([BL, FREE], f32, tag="s_ps")
                        for tb in range(0, T, BATCH):
                            sb = tb // BATCH
                            u_sl = u16[:, tb:tb + BATCH, :]
                            # big bf16 elementwise ops go through
                            # scalar_tensor_tensor: TSP supports the DVE 4x
                            # perf mode (TENSOR_TENSOR caps at 2x)
                            vu = rp.tile([128, BATCH, FREE], bf16, tag="vu")
                            nc.vector.tensor_mul(
                                vu[:], u_sl,
                                vexp1[:].unsqueeze(1).broadcast_to(
                                    [128, BATCH, FREE]))
                            # k-tree reduce: in (k,o) layout the k-halves are
                            # contiguous column blocks, so every level is a 3D
                            # AP (TensorScalarPtr requires <=3D)
                            t1 = treep.tile([128, BATCH, 8 * O], bf16, tag="t1")
                            nc.vector.tensor_add(
                                t1[:], vu[:, :, 0:8 * O], vu[:, :, 8 * O:16 * O])
                            # overlay: t2/t3 reuse t1's low half (out == in0
                            # elementwise, identical strides -> no hazard)
                            t2 = t1[:, :, 0:4 * O]
                            nc.vector.tensor_add(
                                t2, t1[:, :, 0:4 * O], t1[:, :, 4 * O:8 * O])
                            t3 = t1[:, :, 0:2 * O]
                            nc.vector.tensor_add(
                                t3, t1[:, :, 0:2 * O], t1[:, :, 2 * O:4 * O])
                            if rnd == 1:
                                lg = bL[:, tb:tb + BATCH, :]
                                nc.vector.tensor_add(
                                    lg, t1[:, :, 0:O], t1[:, :, O:2 * O])
                            else:
                                agr = rp2.tile([128, BATCH, O], bf16, tag="agr")
                                nc.vector.tensor_add(
                                    agr[:], t1[:, :, 0:O], t1[:, :, O:2 * O])
                                lgt = rp2.tile([128, BATCH, O], f32, tag="lg")
                                nc.vector.tensor_add(
                                    lgt[:], agr[:], bL[:, tb:tb + BATCH, :])
                                lg = lgt[:]
                            # batched softmax over o; 1/Z is folded into the
                            # s-matmul stationary instead of scaling e
                            e8 = rp2.tile([128, BATCH, O], bf16, tag="e")
                            nc.scalar.activation(e8[:], lg, ACTF.Exp)
                            z8 = rp2.tile([128, BATCH], f32, tag="z")
                            nc.vector.reduce_sum(z8[:], e8[:], axis=AX.X)
                            rz8 = rp2.tile([128, BATCH], f32, tag="rz")
                            nc.vector.reciprocal(rz8[:], z8[:])
                            # per-tile stationary S[:, j, :] = ones * rz[:, j]
                            s8 = rp2.tile([128, BATCH, BL], bf16, tag="s8")
                            nc.vector.tensor_mul(
                                s8[:],
                                ones_sb[:].unsqueeze(1).broadcast_to(
                                    [128, BATCH, BL]),
                                rz8[:].unsqueeze(2).broadcast_to(
                                    [128, BATCH, BL]))
                            # eu = u * e (unnormalized; 1/Z is in the
                            # stationary). The k-broadcast makes this 4D, so
                            # TensorScalarPtr can't be used; TENSOR_TENSOR 2x.
                            eu = rp.tile([128, BATCH, K, O], bf16, tag="vu")
                            nc.vector.tensor_mul(
                                eu[:], u_sl.rearrange("p t (k o) -> p t k o", o=O),
                                e8[:].unsqueeze(2).broadcast_to(
                                    [128, BATCH, K, O]))
                            # s += sum_i (1/Z)*eu  (PE partition reduce)
                            for j in range(BATCH):
                                t = tb + j
                                nc.tensor.matmul(
                                    s_ps[:], s8[:, j, :],
                                    eu[:, j, :, :].rearrange("p k o -> p (k o)"),
                                    start=(t == 0), stop=(t == T - 1))
                        vexp1 = squash_and_bcast(s_ps, 1.0, last=(rnd == 2))
    nc.finalize()
    return nc


def _host_prep():
    """Core-independent input prep pieces."""
    ones = np.zeros((128, BL), dtype=BF16)
    for g in range(G):
        for b in range(BL):
            ones[g * 8 + b, b] = 1
    onest = np.ascontiguousarray(ones.T)
    mask = np.zeros((128, 128), dtype=BF16)
    for g in range(G):
        mask[g * 8:(g + 1) * 8, g * 8:(g + 1) * 8] = 1
    return ones, onest, mask


def kernel(x: np.ndarray, W: np.ndarray) -> np.ndarray:
    from concourse import bass_utils

    if "nc" not in _CACHE:
        _CACHE["nc"] = _build_bass()
        _CACHE["ones"], _CACHE["onest"], _CACHE["mask"] = _host_prep()
    nc = _CACHE["nc"]

    # W -> [T, (g,d), (k,o)] : w[t, g*8+d, k*32+o] = W[t*16+g, o, d, k]
    wr = (W.reshape(T, G, O, D, K).transpose(0, 1, 3, 4, 2)
          .reshape(T, 128, FREE).astype(BF16))
    # 4 tiles per DMA: [T//WQ, 128, WQ, FREE]
    wr4 = np.ascontiguousarray(
        wr.reshape(T // WQ, WQ, 128, FREE).transpose(0, 2, 1, 3))
    in_maps = []
    for c in range(NC_N):
        xl = x[c * BL:(c + 1) * BL]  # [8, 2048, 8]
        # xt[g*8+d, t, b] = xl[b, t*16+g, d]
        xt = np.ascontiguousarray(
            xl.reshape(BL, T, G, D).transpose(2, 3, 1, 0).reshape(128, T, BL)
        ).astype(BF16)
        in_maps.append({"w": wr4, "xt": xt, "mask": _CACHE["mask"],
                        "ones": _CACHE["ones"], "onest": _CACHE["onest"]})

    _CACHE["in_maps"] = in_maps
    res = bass_utils.run_bass_kernel_spmd(nc, in_maps, core_ids=list(range(NC_N)))
    out = np.empty((B, O, K), np.float32)
    for c in range(NC_N):
        v = res.results[c]["out"].reshape(BL, K, O)  # (k,o) cols
        out[c * BL:(c + 1) * BL] = v.transpose(0, 2, 1)
    return out
